# revision 4
# baseline (speedup 1.0000x reference)
"""Trainium2 Bass kernel for Falcon-7B MQA flash-decode attention block.

Geometry (hardcoded from the problem spec):
  hidden [1, 32, 4544], w_qkv [4672, 4544] (71 q heads + 1 k + 1 v, hd=64),
  kv cache [4, 1, 32, 2048, 64], masks [4, 1, 32, 2048], w_dense [4544, 4544].

Sharding across 8 NeuronCores:
  - users (32) are data-parallel, 4 per core: each core holds its users' KV.
  - w_qkv / w_dense are tensor-parallel column-split 8 ways; an AllToAll
    redistributes the fused QKV activations from column-shards to user-shards,
    and an AllGather collects attention outputs for the dense matmul.
  - softmax uses the shift-invariant (max-free) formulation, which is exact
    for these magnitudes in fp32; masks enter through the ACT exp bias.

Host-side prep is layout-only (transposes / slicing / padding of inputs).
"""

import sys

if "/opt/trn_rl_repo" not in sys.path:
    sys.path.insert(0, "/opt/trn_rl_repo")

import numpy as np

import concourse.bacc as bacc
import concourse.bass as bass
import concourse.mybir as mybir
import concourse.tile as tile
from concourse.bass_utils import run_bass_kernel_spmd
from concourse.masks import make_identity

F32 = mybir.dt.float32
# float32r: same fp32 bits, but the PE streams the moving operand at 1
# cycle/row when the free dim is >=256 (vs 4 for plain fp32). Verified
# against the fp32 reference on hardware before being enabled here.
F32R = mybir.dt.float32r

NCORES = 8
U = 32          # users total
UPC = 4         # users per core
HID = 4544
NH = 71         # query heads
HD = 64
HPC = 10        # heads per core in the padded qkv column split (8*10*64 = 5120)
NCOL = HPC * HD         # 640 fused columns per core
DN = HID // NCORES      # 568 dense output columns per core
S = 8192                # total cached tokens per user (4 chunks x 2048)
NT = S // 128           # 64 s-tiles of 128
NTH = NT // 2           # 32 tiles per kT partition-half
KT = 36                 # k-tiles over HID: 35 x 128 + 1 x 64
ROWS_FULL = 35 * 128    # 4480

LAST_RESULT = None
_prog = None


def _build():
    nc = bacc.Bacc("TRN2", target_bir_lowering=False, debug=False,
                   num_devices=NCORES)

    hT = nc.dram_tensor("hT", [HID, U], F32R, kind="ExternalInput")
    wq = nc.dram_tensor("wq", [HID, NCOL], F32R, kind="ExternalInput")
    wd = nc.dram_tensor("wd", [HID, DN], F32R, kind="ExternalInput")
    kTc = nc.dram_tensor("kTc", [UPC, 128, S // 2], F32, kind="ExternalInput")
    vc = nc.dram_tensor("vc", [UPC, S, HD], F32, kind="ExternalInput")
    mc = nc.dram_tensor("mc", [UPC, NT, 128], F32, kind="ExternalInput")
    # MuT[i] = (diag(cos_u) + diag(sin_u) @ R)^T per local user, R = rotate_half
    muT = nc.dram_tensor("muT", [HD, UPC, HD], F32, kind="ExternalInput")
    outc = nc.dram_tensor("outc", [U, DN], F32, kind="ExternalOutput")

    with tile.TileContext(nc) as tc:
        with (
            tc.tile_pool(name="const", bufs=1) as const,
            tc.tile_pool(name="wpool", bufs=2) as wpool,
            tc.tile_pool(name="wdpool", bufs=3) as wdpool,
            tc.tile_pool(name="kvpool", bufs=2) as kvpool,
            tc.tile_pool(name="upool", bufs=2) as upool,
            tc.tile_pool(name="ppool", bufs=2) as ppool,
            tc.tile_pool(name="pspool", bufs=4, space="PSUM") as pspool,
            tc.tile_pool(name="pvpool", bufs=2, space="PSUM") as pvpool,
            tc.tile_pool(name="pstpool", bufs=2, space="PSUM") as pstpool,
            tc.tile_pool(name="dram", bufs=1, space="DRAM") as dram,
        ):
            identity = const.tile([128, 128], F32)
            make_identity(nc, identity)

            # ---------------- phase A: fused QKV projection ----------------
            hT_all = const.tile([128, KT, U], F32R)
            nc.sync.dma_start(
                out=hT_all[:, 0:35, :],
                in_=hT[0:ROWS_FULL, :].rearrange("(t p) u -> p t u", p=128))
            nc.sync.dma_start(out=hT_all[0:64, 35, :], in_=hT[ROWS_FULL:HID, :])

            muT_sb = const.tile([HD, UPC, HD], F32)
            nc.sync.dma_start(out=muT_sb, in_=muT[:, :, :])

            psA = pspool.tile([U, 512], F32, tag="bank", name="psA")
            psB = pspool.tile([U, NCOL - 512], F32, tag="bank", name="psB")
            for g in range(7):
                wslab = wpool.tile([128, 5, NCOL], F32R, tag="w", name="wslab")
                nc.sync.dma_start(
                    out=wslab,
                    in_=wq[g * 640:(g + 1) * 640, :].rearrange(
                        "(t p) n -> p t n", p=128))
                for t5 in range(5):
                    t = 5 * g + t5
                    lhs = hT_all[:, t, :]
                    nc.tensor.matmul(psA, lhs, wslab[:, t5, 0:512],
                                     start=(t == 0), stop=False)
                    nc.tensor.matmul(psB, lhs, wslab[:, t5, 512:NCOL],
                                     start=(t == 0), stop=False)
            wlast = wpool.tile([64, NCOL], F32R, tag="wl", name="wlast")
            nc.sync.dma_start(out=wlast, in_=wq[ROWS_FULL:HID, :])
            nc.tensor.matmul(psA, hT_all[0:64, 35, :], wlast[:, 0:512],
                             start=False, stop=True)
            nc.tensor.matmul(psB, hT_all[0:64, 35, :], wlast[:, 512:NCOL],
                             start=False, stop=True)

            fused_sb = const.tile([U, NCOL], F32)
            nc.vector.tensor_copy(out=fused_sb[:, 0:512], in_=psA[:, :])
            nc.vector.tensor_copy(out=fused_sb[:, 512:NCOL], in_=psB[:, :])

            fused_x = dram.tile([U, NCOL], F32)
            nc.sync.dma_start(out=fused_x, in_=fused_sb)
            # block d of the flat input (users 4d..4d+3) goes to core d
            fused_loc = dram.tile([NCORES, UPC, NCOL], F32)
            nc.gpsimd.collective_compute(
                "AllToAll", mybir.AluOpType.bypass,
                replica_groups=[list(range(NCORES))],
                ins=[fused_x.opt()], outs=[fused_loc.opt()])

            # ---------------- phase C: per-user flash-decode attention ------
            attn_c = dram.tile([UPC, HID], F32)

            wd_slabs = []

            def _emit_wd_slab(g):
                wdslab = wdpool.tile([128, 5, DN], F32R, tag="w",
                                     name="wdslab", uniquify=True)
                nc.sync.dma_start(
                    out=wdslab,
                    in_=wd[g * 640:(g + 1) * 640, :].rearrange(
                        "(t p) n -> p t n", p=128))
                wd_slabs.append(wdslab)

            for i in range(UPC):
                kT_sb = kvpool.tile([128, S // 2], F32, tag="kT", name="kT_sb")
                nc.sync.dma_start(out=kT_sb, in_=kTc[i])
                vones = kvpool.tile([128, NT, HD + 1], F32, tag="v",
                                    name="vones")
                nc.sync.dma_start(
                    out=vones[:, :, 0:HD],
                    in_=vc[i].rearrange("(t p) d -> p t d", p=128))
                nc.vector.memset(vones[:, :, HD:HD + 1], 1.0)

                mask_raw = upool.tile([NT, 128], F32, tag="mraw",
                                      name="mask_raw")
                nc.sync.dma_start(out=mask_raw, in_=mc[i])
                ps_m = pstpool.tile([128, NT], F32, tag="pst", name="ps_m")
                nc.tensor.transpose(ps_m, mask_raw, identity[0:NT, 0:NT])
                mask_sb = upool.tile([128, NT], F32, tag="msb", name="mask_sb")
                nc.vector.tensor_copy(out=mask_sb, in_=ps_m)

                # q heads 0..70 plus the shared k head at row 71
                q_raw = upool.tile([80, HD], F32, tag="qraw", name="q_raw")
                for c in range(NCORES):
                    nc.sync.dma_start(
                        out=q_raw[c * HPC:(c + 1) * HPC, :],
                        in_=fused_loc[c, i, :].rearrange("(h d) -> h d", d=HD))
                ps_qT = pstpool.tile([HD, NH + 1], F32, tag="pst",
                                     name="ps_qT")
                nc.tensor.transpose(ps_qT, q_raw[0:NH + 1, :],
                                    identity[0:NH + 1, 0:NH + 1])
                qkT = upool.tile([HD, NH + 1], F32, tag="qkT", name="qkT")
                nc.vector.tensor_copy(out=qkT, in_=ps_qT)

                # rotary as a matmul; duplicated to partitions 64..127 so the
                # second kT half can use it as a same-base moving operand
                ps_rot = pstpool.tile([128, NH + 1], F32, tag="pst",
                                      name="ps_rot")
                nc.tensor.matmul(ps_rot[0:64, :], muT_sb[:, i, :], qkT,
                                 start=True, stop=True)
                nc.tensor.matmul(ps_rot[64:128, :], muT_sb[:, i, :], qkT,
                                 start=True, stop=True)
                qTr = upool.tile([128, NH + 1], F32, tag="qTr", name="qTr")
                nc.vector.tensor_copy(out=qTr, in_=ps_rot)

                vc1 = upool.tile([1, HD + 1], F32, tag="vc1", name="vc1")
                nc.sync.dma_start(
                    out=vc1[:, 0:HD],
                    in_=fused_loc[7, i, 2 * HD:3 * HD][None, :])
                nc.vector.memset(vc1[:, HD:HD + 1], 1.0)

                # scores^T + exp for all 64 s-tiles
                pT_all = ppool.tile([128, NT, NH], F32, tag="pT",
                                    name="pT_all")
                for jp in range(NTH):
                    for j in (jp, jp + NTH):
                        if j < NTH:
                            lhsT = kT_sb[0:64, j * 128:(j + 1) * 128]
                            rhs = qTr[0:64, 0:NH]
                        else:
                            lhsT = kT_sb[64:128,
                                         (j - NTH) * 128:(j - NTH + 1) * 128]
                            rhs = qTr[64:128, 0:NH]
                        ps_s = pspool.tile([128, NH], F32, tag="bank",
                                           name="ps_s")
                        nc.tensor.matmul(ps_s, lhsT, rhs, start=True,
                                         stop=True)
                        nc.scalar.activation(
                            out=pT_all[:, j, :], in_=ps_s,
                            func=mybir.ActivationFunctionType.Exp,
                            bias=mask_sb[:, j:j + 1], scale=0.125)

                # current-token score for all heads: [1, 71]
                ps_sc = pstpool.tile([1, NH], F32, tag="pst", name="ps_sc")
                nc.tensor.matmul(ps_sc, qTr[0:64, NH:NH + 1], qTr[0:64, 0:NH],
                                 start=True, stop=True)
                curw = upool.tile([1, NH], F32, tag="curw", name="curw")
                nc.scalar.activation(out=curw, in_=ps_sc,
                                     func=mybir.ActivationFunctionType.Exp,
                                     scale=0.125)

                # PV with fused row-sum via the ones column
                pv = pvpool.tile([NH, HD + 1], F32, tag="pv", name="pv")
                for j in range(NT):
                    nc.tensor.matmul(pv, pT_all[:, j, :], vones[:, j, :],
                                     start=(j == 0), stop=False)
                nc.tensor.matmul(pv, curw, vc1, start=False, stop=True)

                linv = upool.tile([NH, 1], F32, tag="linv", name="linv")
                nc.vector.reciprocal(out=linv, in_=pv[:, HD:HD + 1])
                attn_sb = upool.tile([NH, HD], F32, tag="attn", name="attn_sb")
                nc.vector.tensor_scalar_mul(attn_sb, pv[:, 0:HD], linv)
                nc.sync.dma_start(
                    out=attn_c[i].rearrange("(h d) -> h d", d=HD),
                    in_=attn_sb)

                if i < 3:
                    _emit_wd_slab(i)

            attn_ag = dram.tile([NCORES, UPC, HID], F32, addr_space="Shared")
            nc.gpsimd.collective_compute(
                "AllGather", mybir.AluOpType.bypass,
                replica_groups=[list(range(NCORES))],
                ins=[attn_c.opt()], outs=[attn_ag.opt()])

            # ---------------- phase D: dense output projection --------------
            attn_flat = attn_ag.rearrange("c i n -> (c i) n")
            attnT_all = const.tile([128, KT, U], F32R)
            for t in range(KT):
                cw = 128 if t < 35 else 64
                a_chunk = upool.tile([U, 128], F32, tag="achunk",
                                     name="a_chunk")
                nc.sync.dma_start(out=a_chunk[:, 0:cw],
                                  in_=attn_flat[:, t * 128:t * 128 + cw])
                ps_t2 = pstpool.tile([128, U], F32, tag="pst", name="ps_t2")
                nc.tensor.transpose(ps_t2[0:cw, :], a_chunk[:, 0:cw],
                                    identity[0:U, 0:U])
                nc.vector.tensor_copy(out=attnT_all[0:cw, t, :],
                                      in_=ps_t2[0:cw, :])

            psD1 = pspool.tile([U, 512], F32, tag="bank", name="psD1")
            psD2 = pspool.tile([U, DN - 512], F32, tag="bank", name="psD2")
            for g in range(7):
                if g < len(wd_slabs):
                    wdslab = wd_slabs[g]
                else:
                    _emit_wd_slab(g)
                    wdslab = wd_slabs[g]
                for t5 in range(5):
                    t = 5 * g + t5
                    lhs = attnT_all[:, t, :]
                    nc.tensor.matmul(psD1, lhs, wdslab[:, t5, 0:512],
                                     start=(t == 0), stop=False)
                    nc.tensor.matmul(psD2, lhs, wdslab[:, t5, 512:DN],
                                     start=(t == 0), stop=False)
            wdlast = wpool.tile([64, DN], F32R, tag="wl", name="wdlast")
            nc.sync.dma_start(out=wdlast, in_=wd[ROWS_FULL:HID, :])
            nc.tensor.matmul(psD1, attnT_all[0:64, 35, :], wdlast[:, 0:512],
                             start=False, stop=True)
            nc.tensor.matmul(psD2, attnT_all[0:64, 35, :], wdlast[:, 512:DN],
                             start=False, stop=True)

            outD = const.tile([U, DN], F32)
            nc.vector.tensor_copy(out=outD[:, 0:512], in_=psD1[:, :])
            nc.vector.tensor_copy(out=outD[:, 512:DN], in_=psD2[:, :])
            nc.sync.dma_start(out=outc[:, :], in_=outD)

    nc.compile()
    return nc


def _rot_mat(cos_u, sin_u):
    """M such that M @ x = x*cos + rotate_half(x)*sin, for one user."""
    m = np.zeros((HD, HD), np.float32)
    np.fill_diagonal(m, cos_u)
    half = HD // 2
    for r in range(half):
        m[r, r + half] += -sin_u[r]
        m[r + half, r] += sin_u[r + half]
    return m


def kernel(hidden_states, cos, sin, k_cache, v_cache, attn_masks, w_qkv,
           w_dense, trace=False):
    global _prog, LAST_RESULT
    if _prog is None:
        _prog = _build()

    hidden_states = np.asarray(hidden_states, np.float32)
    cos = np.asarray(cos, np.float32)
    sin = np.asarray(sin, np.float32)
    k_cache = np.asarray(k_cache, np.float32)
    v_cache = np.asarray(v_cache, np.float32)
    attn_masks = np.asarray(attn_masks, np.float32)
    w_qkv = np.asarray(w_qkv, np.float32)
    w_dense = np.asarray(w_dense, np.float32)

    hT = np.ascontiguousarray(hidden_states[0].T)            # [4544, 32]
    wqT = np.zeros((HID, NCORES * NCOL), np.float32)
    wqT[:, :w_qkv.shape[0]] = w_qkv.T
    wdT = np.ascontiguousarray(w_dense.T)                    # [4544, 4544]

    in_maps = []
    for c in range(NCORES):
        us = slice(UPC * c, UPC * (c + 1))
        k_u = np.moveaxis(k_cache[:, 0, us], 1, 0).reshape(UPC, S, HD)
        kT_u = np.transpose(k_u, (0, 2, 1))                  # [4, 64, 8192]
        kT_pack = np.concatenate(
            [kT_u[:, :, :S // 2], kT_u[:, :, S // 2:]], axis=1)
        v_u = np.moveaxis(v_cache[:, 0, us], 1, 0).reshape(UPC, S, HD)
        m_u = np.moveaxis(attn_masks[:, 0, us], 1, 0).reshape(UPC, NT, 128)
        muT = np.stack([
            _rot_mat(cos[0, u, 0], sin[0, u, 0]).T
            for u in range(UPC * c, UPC * (c + 1))
        ])                                                   # [4, 64, 64]
        in_maps.append({
            "hT": hT,
            "wq": np.ascontiguousarray(wqT[:, NCOL * c:NCOL * (c + 1)]),
            "wd": np.ascontiguousarray(wdT[:, DN * c:DN * (c + 1)]),
            "kTc": np.ascontiguousarray(kT_pack),
            "vc": np.ascontiguousarray(v_u),
            "mc": np.ascontiguousarray(m_u),
            "muT": np.ascontiguousarray(np.transpose(muT, (1, 0, 2))),
        })

    res = run_bass_kernel_spmd(_prog, in_maps, list(range(NCORES)),
                               trace=trace)
    LAST_RESULT = res
    out = np.concatenate([res.results[c]["outc"] for c in range(NCORES)],
                         axis=1)                             # [32, 4544]
    return out[None].astype(np.float32)
